# revision 14
# baseline (speedup 1.0000x reference)
"""Low-rank (CPD) 3D conv kernel for Trainium2, SPMD across 8 NeuronCores.

Math (per reference):
  y[r,h,w,d]  = sum_c U_c_in[c,r] * x[c,h,w,d]
  z           = conv_h(conv_w(conv_d(y)))   (separable 3-tap, per-rank taps)
  out[c,...]  = sum_r U_c_out[r,c] * z[r,...] + bias[c]

Distribution: data-parallel split of H (64) into 8 slabs of 8 planes; each
core reads its slab plus one halo plane on each side (zero at global edges)
and computes its output slab independently. No collectives.

Per-core pipeline (software-pipelined over planes, p = h + 3):
  - mm1 per INPUT plane (1x flops; the old kernel folded conv_h here at 3x):
    y[p] = U_c_in^T x[p], PSUM accumulated over 2 c-tiles, ACT-drained
    dense to bf16.
  - conv_h on DVE: 2 fused scalar_tensor_tensor passes per rank-tile using
    tap ratios U0/U1, U2/U1 (the U1 scale is folded into the mm2 weights),
    full-plane aligned -> 2x DVE mode.
  - conv_w on DVE: same 2-pass STT trick with +-1 w-line shifts (aligned),
    writing into a zero-padded z layout (66-wide d-lines, data in [0:64),
    pads stay zero) so mm2 can read d-shifted views safely.
  - conv_d is folded into mm2: out = sum_{k,rt} W_k[rt] @ z_shift(k) where
    W_k = U_kh[1]*U_kw[1]*U_kd[k] * U_c_out and z_shift(k) is a strided AP
    at element offset k-1 into the padded z lines (PE reads are
    alignment-insensitive; the padding zeros implement d-edge zero-pad).
  - mm2 drain on ACT with per-partition bias, bf16 output (host upcasts to
    f32), halving output DMA.
"""

import numpy as np
import ml_dtypes

BF16 = ml_dtypes.bfloat16

# Problem constants (hardcoded per contest contract)
C = 256   # input channels
R = 256   # rank
CO = 256  # output channels
S = 64    # spatial extent (cube)
NCORES = 8
HP = S // NCORES          # output planes per core (8)
HS = HP + 2               # slab planes incl. halo (10)
PLANE = S * S             # 4096 elements per (w,d) plane
ZLINE = S + 2             # padded d-line length (66)
ZPAD = 2 + ZLINE * S + 2  # padded z tile free dim (guards + 64 lines)

_cache = {}


def _build_program(hp=HP):
    """Build and compile the per-core Bass program (identical on all cores)."""
    import concourse.bass as bass
    import concourse.mybir as mybir
    import concourse.tile as tile
    from concourse import bacc

    HS_, HP_ = hp + 2, hp

    fp32 = mybir.dt.float32
    bf16 = mybir.dt.bfloat16
    mult = mybir.AluOpType.mult
    add = mybir.AluOpType.add
    ident = mybir.ActivationFunctionType.Identity

    nc = bacc.Bacc("TRN2", target_bir_lowering=False, debug=False,
                   num_devices=NCORES)

    # DRAM tensors (names are the in_map keys)
    x_d = nc.dram_tensor("xs", [2, 128, HS_, PLANE], bf16, kind="ExternalInput").ap()
    w1_d = nc.dram_tensor("w1", [2, 2, 128, 128], bf16, kind="ExternalInput").ap()
    w2_d = nc.dram_tensor("w2", [3, 2, 2, 128, 128], bf16, kind="ExternalInput").ap()
    rh_d = nc.dram_tensor("rh", [2, 128, 2], fp32, kind="ExternalInput").ap()
    rw_d = nc.dram_tensor("rw", [2, 128, 2], fp32, kind="ExternalInput").ap()
    bias_d = nc.dram_tensor("bias_t", [2, 128, 1], fp32, kind="ExternalInput").ap()
    out_d = nc.dram_tensor("out", [2, 128, HP_, PLANE], bf16, kind="ExternalOutput").ap()

    with tile.TileContext(nc) as tc:
        consts = tc.alloc_tile_pool(name="consts", bufs=1)
        xpool = tc.alloc_tile_pool(name="x", bufs=4)
        ypool = tc.alloc_tile_pool(name="y", bufs=8)
        tpool = tc.alloc_tile_pool(name="tmp", bufs=4)
        gpool = tc.alloc_tile_pool(name="gtmp", bufs=4)
        zpool = tc.alloc_tile_pool(name="z", bufs=1)
        opool = tc.alloc_tile_pool(name="osb", bufs=2)
        ps = tc.alloc_tile_pool(name="ps", bufs=2, space="PSUM")

        # ---- x plane streaming (x(0) DMA first: it gates mm1(0)) ----
        xt = {}

        def get_x(p, ct):
            if (p, ct) not in xt:
                t = xpool.tile([128, PLANE], bf16, name="xplane", tag="xplane")
                nc.sync.dma_start(out=t, in_=x_d[ct, :, p, :])
                xt[(p, ct)] = t
            return xt[(p, ct)]

        for ct in range(2):
            get_x(0, ct)

        # ---- constants ----
        w1 = [[consts.tile([128, 128], bf16, name=f"w1_{ct}{rt}", tag=f"w1_{ct}{rt}")
               for rt in range(2)] for ct in range(2)]
        for ct in range(2):
            for rt in range(2):
                nc.sync.dma_start(out=w1[ct][rt], in_=w1_d[ct, rt])
        w2 = [[[consts.tile([128, 128], bf16, name=f"w2_{k}{rt}{co}", tag=f"w2_{k}{rt}{co}")
                for co in range(2)] for rt in range(2)] for k in range(3)]
        for k in range(3):
            for rt in range(2):
                for co in range(2):
                    nc.sync.dma_start(out=w2[k][rt][co], in_=w2_d[k, rt, co])
        rh = [consts.tile([128, 2], fp32, name=f"rh{rt}", tag=f"rh{rt}") for rt in range(2)]
        rw = [consts.tile([128, 2], fp32, name=f"rw{rt}", tag=f"rw{rt}") for rt in range(2)]
        bia = [consts.tile([128, 1], fp32, name=f"bias{co}", tag=f"bias{co}") for co in range(2)]
        for rt in range(2):
            nc.sync.dma_start(out=rh[rt], in_=rh_d[rt])
            nc.sync.dma_start(out=rw[rt], in_=rw_d[rt])
        for co in range(2):
            nc.sync.dma_start(out=bia[co], in_=bias_d[co])

        # ---- persistent padded z tiles (pads memset once, stay zero) ----
        zt = {}
        for slot in range(2):
            for rt in range(2):
                t = zpool.tile([128, ZPAD], bf16, name=f"zt{slot}{rt}",
                               tag=f"zt{slot}{rt}")
                # only guards + per-line pad slots need zeroing
                nc.vector.memset(t[:, 0:2], 0.0)
                nc.vector.memset(t[:, ZPAD - 2:ZPAD], 0.0)
                nc.vector.memset(
                    t[:, 2:2 + ZLINE * S].rearrange(
                        "p (w e) -> p w e", e=ZLINE)[:, :, S:ZLINE], 0.0)
                zt[(slot, rt)] = t

        def zlines(slot, rt):
            # [128, 64 lines, 64 data] view of the padded z tile
            return zt[(slot, rt)][:, 2:2 + ZLINE * S].rearrange(
                "p (w e) -> p w e", e=ZLINE)[:, :, 0:S]

        def zrhs(slot, rt, q, k):
            # mm2 moving operand: 8 w-lines x 64 cols at d-offset (k-1)
            b = 2 + ZLINE * (8 * q) + (k - 1)
            return zt[(slot, rt)][:, b:b + 8 * ZLINE].rearrange(
                "p (w e) -> p w e", e=ZLINE)[:, :, 0:S]

        yt = {}  # (p%4, rt) -> dense bf16 y tile

        def mm1(p):
            for rt in range(2):
                if (p % 4, rt) not in yt:
                    yt[(p % 4, rt)] = ypool.tile([128, PLANE], bf16,
                                                 name="yplane", tag="yplane")
                ysb = yt[(p % 4, rt)]
                for half in range(2):
                    pt = ps.tile([128, 2048], fp32, name="pt", tag="ps")
                    for c4 in range(4):
                        q = half * 4 + c4
                        for ct in range(2):
                            nc.tensor.matmul(
                                pt[:, c4 * 512:(c4 + 1) * 512],
                                w1[ct][rt],
                                get_x(p, ct)[:, q * 512:(q + 1) * 512],
                                start=(ct == 0),
                                stop=(ct == 1),
                            )
                    nc.scalar.copy(ysb[:, half * 2048:(half + 1) * 2048], pt)

        def conv(h):
            """conv_h + conv_w for out-plane h -> padded z[(h%2, rt)].

            STT measured 1x on HW (scalar AP costs a read port), so each
            pass is TS (4x) + TT (2x). rt1 runs first and its final z-pass
            goes to GpSimd: that output is consumed only by PE's mm2 next
            phase, so the slow engine has a full phase of slack and never
            blocks the DVE queue. Its inputs live in a dedicated pool so
            DVE tmp-slot recycling doesn't wait on GpSimd either.
            """
            slot = h % 2
            y = {(i, rt): yt[((h + i) % 4, rt)]
                 for i in range(3) for rt in range(2)}
            for rt in (1, 0):
                pool = gpool if rt == 1 else tpool
                tg = "gtmp" if rt == 1 else "tmp"
                sy0 = tpool.tile([128, PLANE], bf16, name="sy0", tag="tmp")
                nc.vector.tensor_scalar_mul(sy0, y[(0, rt)], rh[rt][:, 0:1])
                th = tpool.tile([128, PLANE], bf16, name="th", tag="tmp")
                nc.vector.tensor_tensor(th, sy0, y[(1, rt)], add)
                sy2 = tpool.tile([128, PLANE], bf16, name="sy2", tag="tmp")
                nc.vector.tensor_scalar_mul(sy2, y[(2, rt)], rh[rt][:, 1:2])
                a = tpool.tile([128, PLANE], bf16, name="ah", tag="tmp")
                nc.vector.tensor_tensor(a, sy2, th, add)
                av = a.rearrange("p (w q) -> p w q", q=S)

                # conv_w: t2[w] = r0w*a[w-1] + a[w]; z = r2w*a[w+1] + t2
                sa = tpool.tile([128, PLANE], bf16, name="sa", tag="tmp")
                nc.vector.tensor_scalar_mul(sa, a, rw[rt][:, 0:1])
                sav = sa.rearrange("p (w q) -> p w q", q=S)
                t2 = pool.tile([128, PLANE], bf16, name="t2", tag=tg)
                t2v = t2.rearrange("p (w q) -> p w q", q=S)
                nc.vector.tensor_tensor(
                    t2v[:, 1:, :], sav[:, :-1, :], av[:, 1:, :], add)
                nc.vector.tensor_copy(t2v[:, 0, :], av[:, 0, :])
                sa2 = pool.tile([128, PLANE], bf16, name="sa2", tag=tg)
                nc.vector.tensor_scalar_mul(sa2, a, rw[rt][:, 1:2])
                sa2v = sa2.rearrange("p (w q) -> p w q", q=S)
                zv = zlines(slot, rt)
                eng = nc.gpsimd if rt == 1 else nc.vector
                eng.tensor_tensor(
                    zv[:, 0:S - 1, :], sa2v[:, 1:, :], t2v[:, 0:S - 1, :], add)
                eng.tensor_copy(zv[:, S - 1, :], t2v[:, S - 1, :])

        def mm2(h):
            slot = h % 2
            for co in range(2):
                for half in range(2):
                    pt = ps.tile([128, 2048], fp32, name="pt2", tag="ps")
                    for c4 in range(4):
                        q = half * 4 + c4
                        n = 0
                        for rt in range(2):
                            for k in range(3):
                                nc.tensor.matmul(
                                    pt[:, c4 * 512:(c4 + 1) * 512],
                                    w2[k][rt][co],
                                    zrhs(slot, rt, q, k),
                                    start=(n == 0),
                                    stop=(n == 5),
                                )
                                n += 1
                    osb = opool.tile([128, 2048], bf16, name="osb", tag="osb")
                    nc.scalar.activation(osb, pt, ident, bias=bia[co][:, 0:1])
                    nc.sync.dma_start(
                        out=out_d[co, :, h, half * 2048:(half + 1) * 2048],
                        in_=osb)

        # --- software pipeline: phase h issues mm1(h+4), conv(h+1), mm2(h)
        # so PE's mm2 only depends on the PREVIOUS phase's DVE output.
        for p in range(4):
            for ct in range(2):
                get_x(p, ct)
        mm1(0)
        mm1(1)
        mm1(2)
        conv(0)
        mm1(3)

        for h in range(HP_):
            p = h + 4
            if p < HS_:
                for ct in range(2):
                    get_x(p, ct)
                mm1(p)
            if h + 1 < HP_:
                conv(h + 1)
            mm2(h)

        for pool in (ps, opool, zpool, gpool, tpool, ypool, xpool, consts):
            pool.release()

    nc.compile()
    return nc


def _host_prep(x, U_kh, U_kw, U_kd, U_c_in, U_c_out, bias):
    """Build per-core input maps (numpy only)."""
    x = np.asarray(x)
    U_kh = np.asarray(U_kh, np.float32)
    U_kw = np.asarray(U_kw, np.float32)
    U_kd = np.asarray(U_kd, np.float32)
    U_c_in = np.asarray(U_c_in, np.float32)
    U_c_out = np.asarray(U_c_out, np.float32)
    bias = np.asarray(bias, np.float32)

    xb = np.ascontiguousarray(x[0]).astype(BF16)          # [C, S, S, S]
    xb = xb.reshape(C, S, PLANE)

    # mm1 weights: U_c_in blocks [ct, rt, 128, 128]
    w1 = np.ascontiguousarray(
        U_c_in.astype(BF16).reshape(2, 128, 2, 128).transpose(0, 2, 1, 3))

    # mm2 weights with conv_d taps + U1h*U1w rescale folded in:
    # W_k[r, co] = U_kh[1,r]*U_kw[1,r]*U_kd[k,r]*U_c_out[r,co]
    w2 = np.empty((3, 2, 2, 128, 128), BF16)
    scale_r = U_kh[1] * U_kw[1]                            # [R]
    for k in range(3):
        wk = (scale_r * U_kd[k])[:, None] * U_c_out        # [R, CO]
        w2[k] = wk.astype(BF16).reshape(2, 128, 2, 128).transpose(0, 2, 1, 3)

    # tap ratios for the STT conv passes
    rh = np.stack([U_kh[0] / U_kh[1], U_kh[2] / U_kh[1]], axis=1)  # [R, 2]
    rw = np.stack([U_kw[0] / U_kw[1], U_kw[2] / U_kw[1]], axis=1)
    rh = np.ascontiguousarray(rh.reshape(2, 128, 2).astype(np.float32))
    rw = np.ascontiguousarray(rw.reshape(2, 128, 2).astype(np.float32))
    bias_t = np.ascontiguousarray(bias.reshape(2, 128, 1))

    in_maps = []
    for c in range(NCORES):
        slab = np.zeros((C, HS, PLANE), BF16)
        lo, hi = c * HP - 1, c * HP + HP + 1
        s0, s1 = max(lo, 0), min(hi, S)
        slab[:, s0 - lo:HS - (hi - s1)] = xb[:, s0:s1]
        slab = np.ascontiguousarray(slab.reshape(2, 128, HS, PLANE))
        in_maps.append({
            "xs": slab, "w1": w1, "w2": w2, "rh": rh, "rw": rw,
            "bias_t": bias_t,
        })
    return in_maps


def kernel(x, U_kh, U_kw, U_kd, U_c_in, U_c_out, bias, _trace=False):
    from concourse.bass_utils import run_bass_kernel_spmd

    if "nc" not in _cache:
        _cache["nc"] = _build_program()
    nc = _cache["nc"]

    in_maps = _host_prep(x, U_kh, U_kw, U_kd, U_c_in, U_c_out, bias)
    res = run_bass_kernel_spmd(nc, in_maps, core_ids=list(range(NCORES)),
                               trace=_trace)
    _cache["last_result"] = res

    out = np.empty((1, CO, S, S, S), np.float32)
    for c in range(NCORES):
        o = res.results[c]["out"]                        # [2, 128, HP, PLANE] bf16
        out[0, :, c * HP:(c + 1) * HP] = o.astype(np.float32).reshape(CO, HP, S, S)
    return out


# revision 15
# speedup vs baseline: 1.2231x; 1.2231x over previous
"""Low-rank (CPD) 3D conv kernel for Trainium2, SPMD across 8 NeuronCores.

Math (per reference):
  y[r,h,w,d]  = sum_c U_c_in[c,r] * x[c,h,w,d]
  z           = conv_h(conv_w(conv_d(y)))   (separable 3-tap, per-rank taps)
  out[c,...]  = sum_r U_c_out[r,c] * z[r,...] + bias[c]

Distribution: data-parallel split of H (64) into 8 slabs of 8 planes; each
core reads its slab plus one halo plane on each side (zero at global edges)
and computes its output slab independently. No collectives.

Per-core pipeline (software-pipelined over planes, p = h + 3):
  - mm1 per INPUT plane (1x flops; the old kernel folded conv_h here at 3x):
    y[p] = U_c_in^T x[p], PSUM accumulated over 2 c-tiles, ACT-drained
    dense to bf16.
  - conv_h on DVE: 2 fused scalar_tensor_tensor passes per rank-tile using
    tap ratios U0/U1, U2/U1 (the U1 scale is folded into the mm2 weights),
    full-plane aligned -> 2x DVE mode.
  - conv_w on DVE: same 2-pass STT trick with +-1 w-line shifts (aligned),
    writing into a zero-padded z layout (66-wide d-lines, data in [0:64),
    pads stay zero) so mm2 can read d-shifted views safely.
  - conv_d is folded into mm2: out = sum_{k,rt} W_k[rt] @ z_shift(k) where
    W_k = U_kh[1]*U_kw[1]*U_kd[k] * U_c_out and z_shift(k) is a strided AP
    at element offset k-1 into the padded z lines (PE reads are
    alignment-insensitive; the padding zeros implement d-edge zero-pad).
  - mm2 drain on ACT with per-partition bias, bf16 output (host upcasts to
    f32), halving output DMA.
"""

import numpy as np
import ml_dtypes

BF16 = ml_dtypes.bfloat16

# Problem constants (hardcoded per contest contract)
C = 256   # input channels
R = 256   # rank
CO = 256  # output channels
S = 64    # spatial extent (cube)
NCORES = 8
HP = S // NCORES          # output planes per core (8)
HS = HP + 2               # slab planes incl. halo (10)
PLANE = S * S             # 4096 elements per (w,d) plane
ZLINE = S + 2             # padded d-line length (66)
ZPAD = 2 + ZLINE * S + 2  # padded z tile free dim (guards + 64 lines)

_cache = {}


def _build_program(hp=HP):
    """Build and compile the per-core Bass program (identical on all cores)."""
    import concourse.bass as bass
    import concourse.mybir as mybir
    import concourse.tile as tile
    from concourse import bacc

    HS_, HP_ = hp + 2, hp

    fp32 = mybir.dt.float32
    bf16 = mybir.dt.bfloat16
    mult = mybir.AluOpType.mult
    add = mybir.AluOpType.add
    ident = mybir.ActivationFunctionType.Identity

    nc = bacc.Bacc("TRN2", target_bir_lowering=False, debug=False,
                   num_devices=NCORES)

    # DRAM tensors (names are the in_map keys)
    x_d = nc.dram_tensor("xs", [2, 128, HS_, PLANE], bf16, kind="ExternalInput").ap()
    w1_d = nc.dram_tensor("w1", [2, 2, 128, 128], bf16, kind="ExternalInput").ap()
    w2_d = nc.dram_tensor("w2", [3, 2, 2, 128, 128], bf16, kind="ExternalInput").ap()
    rh_d = nc.dram_tensor("rh", [2, 128, 2], fp32, kind="ExternalInput").ap()
    rw_d = nc.dram_tensor("rw", [2, 128, 2], fp32, kind="ExternalInput").ap()
    bias_d = nc.dram_tensor("bias_t", [2, 128, 1], fp32, kind="ExternalInput").ap()
    out_d = nc.dram_tensor("out", [2, 128, HP_, PLANE], bf16, kind="ExternalOutput").ap()

    with tile.TileContext(nc) as tc:
        consts = tc.alloc_tile_pool(name="consts", bufs=1)
        xpool = tc.alloc_tile_pool(name="x", bufs=4)
        ypool = tc.alloc_tile_pool(name="y", bufs=8)
        tpool = tc.alloc_tile_pool(name="tmp", bufs=4)
        gpool = tc.alloc_tile_pool(name="gtmp", bufs=4)
        zpool = tc.alloc_tile_pool(name="z", bufs=1)
        opool = tc.alloc_tile_pool(name="osb", bufs=2)
        ps = tc.alloc_tile_pool(name="ps", bufs=2, space="PSUM")

        # ---- x plane streaming (x(0) DMA first: it gates mm1(0)) ----
        xt = {}

        def get_x(p, ct):
            if (p, ct) not in xt:
                t = xpool.tile([128, PLANE], bf16, name="xplane", tag="xplane")
                nc.sync.dma_start(out=t, in_=x_d[ct, :, p, :])
                xt[(p, ct)] = t
            return xt[(p, ct)]

        for ct in range(2):
            get_x(0, ct)

        # ---- constants ----
        w1 = [[consts.tile([128, 128], bf16, name=f"w1_{ct}{rt}", tag=f"w1_{ct}{rt}")
               for rt in range(2)] for ct in range(2)]
        for ct in range(2):
            for rt in range(2):
                nc.sync.dma_start(out=w1[ct][rt], in_=w1_d[ct, rt])
        w2 = [[[consts.tile([128, 128], bf16, name=f"w2_{k}{rt}{co}", tag=f"w2_{k}{rt}{co}")
                for co in range(2)] for rt in range(2)] for k in range(3)]
        for k in range(3):
            for rt in range(2):
                for co in range(2):
                    nc.sync.dma_start(out=w2[k][rt][co], in_=w2_d[k, rt, co])
        rh = [consts.tile([128, 2], fp32, name=f"rh{rt}", tag=f"rh{rt}") for rt in range(2)]
        rw = [consts.tile([128, 2], fp32, name=f"rw{rt}", tag=f"rw{rt}") for rt in range(2)]
        bia = [consts.tile([128, 1], fp32, name=f"bias{co}", tag=f"bias{co}") for co in range(2)]
        for rt in range(2):
            nc.sync.dma_start(out=rh[rt], in_=rh_d[rt])
            nc.sync.dma_start(out=rw[rt], in_=rw_d[rt])
        for co in range(2):
            nc.sync.dma_start(out=bia[co], in_=bias_d[co])

        # ---- persistent padded z tiles (pads memset once, stay zero) ----
        zt = {}
        for slot in range(2):
            for rt in range(2):
                t = zpool.tile([128, ZPAD], bf16, name=f"zt{slot}{rt}",
                               tag=f"zt{slot}{rt}")
                # only guards + per-line pad slots need zeroing
                nc.vector.memset(t[:, 0:2], 0.0)
                nc.vector.memset(t[:, ZPAD - 2:ZPAD], 0.0)
                nc.vector.memset(
                    t[:, 2:2 + ZLINE * S].rearrange(
                        "p (w e) -> p w e", e=ZLINE)[:, :, S:ZLINE], 0.0)
                zt[(slot, rt)] = t

        def zlines(slot, rt):
            # [128, 64 lines, 64 data] view of the padded z tile
            return zt[(slot, rt)][:, 2:2 + ZLINE * S].rearrange(
                "p (w e) -> p w e", e=ZLINE)[:, :, 0:S]

        def zrhs(slot, rt, q, k):
            # mm2 moving operand: 8 w-lines x 64 cols at d-offset (k-1)
            b = 2 + ZLINE * (8 * q) + (k - 1)
            return zt[(slot, rt)][:, b:b + 8 * ZLINE].rearrange(
                "p (w e) -> p w e", e=ZLINE)[:, :, 0:S]

        yt = {}  # (p%4, rt) -> dense bf16 y tile

        def mm1(p):
            for rt in range(2):
                if (p % 4, rt) not in yt:
                    yt[(p % 4, rt)] = ypool.tile([128, PLANE], bf16,
                                                 name="yplane", tag="yplane")
                ysb = yt[(p % 4, rt)]
                for half in range(2):
                    pt = ps.tile([128, 2048], fp32, name="pt", tag="ps")
                    for c4 in range(4):
                        q = half * 4 + c4
                        for ct in range(2):
                            nc.tensor.matmul(
                                pt[:, c4 * 512:(c4 + 1) * 512],
                                w1[ct][rt],
                                get_x(p, ct)[:, q * 512:(q + 1) * 512],
                                start=(ct == 0),
                                stop=(ct == 1),
                            )
                    nc.scalar.copy(ysb[:, half * 2048:(half + 1) * 2048], pt)

        def conv(h):
            """conv_h + conv_w for out-plane h -> padded z[(h%2, rt)].

            STT measured 1x on HW (scalar AP costs a read port), so each
            pass is TS (4x) + TT (2x). rt1 runs first and its final z-pass
            goes to GpSimd: that output is consumed only by PE's mm2 next
            phase, so the slow engine has a full phase of slack and never
            blocks the DVE queue. Its inputs live in a dedicated pool so
            DVE tmp-slot recycling doesn't wait on GpSimd either.
            """
            slot = h % 2
            y = {(i, rt): yt[((h + i) % 4, rt)]
                 for i in range(3) for rt in range(2)}
            # rt1's conv_h add goes to GpSimd (dense op only — strided ops
            # are pathological there). Its TS input is DVE's first op, so
            # GpSimd starts ~1us in and finishes before DVE's rt0 chain
            # (13us) reaches the dependent rt1 add.
            sy0_1 = gpool.tile([128, PLANE], bf16, name="sy0_1", tag="gtmp")
            nc.vector.tensor_scalar_mul(sy0_1, y[(0, 1)], rh[1][:, 0:1])
            th1 = gpool.tile([128, PLANE], bf16, name="th1", tag="gtmp")
            nc.gpsimd.tensor_tensor(th1, sy0_1, y[(1, 1)], add)
            for rt in (0, 1):
                if rt == 0:
                    sy0 = tpool.tile([128, PLANE], bf16, name="sy0", tag="tmp")
                    nc.vector.tensor_scalar_mul(sy0, y[(0, 0)], rh[0][:, 0:1])
                    th = tpool.tile([128, PLANE], bf16, name="th", tag="tmp")
                    nc.vector.tensor_tensor(th, sy0, y[(1, 0)], add)
                else:
                    th = th1
                sy2 = tpool.tile([128, PLANE], bf16, name="sy2", tag="tmp")
                nc.vector.tensor_scalar_mul(sy2, y[(2, rt)], rh[rt][:, 1:2])
                a = tpool.tile([128, PLANE], bf16, name="ah", tag="tmp")
                nc.vector.tensor_tensor(a, sy2, th, add)
                av = a.rearrange("p (w q) -> p w q", q=S)

                # conv_w: t2[w] = r0w*a[w-1] + a[w]; z = r2w*a[w+1] + t2
                sa = tpool.tile([128, PLANE], bf16, name="sa", tag="tmp")
                nc.vector.tensor_scalar_mul(sa, a, rw[rt][:, 0:1])
                sav = sa.rearrange("p (w q) -> p w q", q=S)
                t2 = tpool.tile([128, PLANE], bf16, name="t2", tag="tmp")
                t2v = t2.rearrange("p (w q) -> p w q", q=S)
                nc.vector.tensor_tensor(
                    t2v[:, 1:, :], sav[:, :-1, :], av[:, 1:, :], add)
                nc.vector.tensor_copy(t2v[:, 0, :], av[:, 0, :])
                sa2 = tpool.tile([128, PLANE], bf16, name="sa2", tag="tmp")
                nc.vector.tensor_scalar_mul(sa2, a, rw[rt][:, 1:2])
                sa2v = sa2.rearrange("p (w q) -> p w q", q=S)
                zv = zlines(slot, rt)
                nc.vector.tensor_tensor(
                    zv[:, 0:S - 1, :], sa2v[:, 1:, :], t2v[:, 0:S - 1, :], add)
                nc.vector.tensor_copy(zv[:, S - 1, :], t2v[:, S - 1, :])

        def mm2(h):
            slot = h % 2
            for co in range(2):
                for half in range(2):
                    pt = ps.tile([128, 2048], fp32, name="pt2", tag="ps")
                    for c4 in range(4):
                        q = half * 4 + c4
                        n = 0
                        for rt in range(2):
                            for k in range(3):
                                nc.tensor.matmul(
                                    pt[:, c4 * 512:(c4 + 1) * 512],
                                    w2[k][rt][co],
                                    zrhs(slot, rt, q, k),
                                    start=(n == 0),
                                    stop=(n == 5),
                                )
                                n += 1
                    osb = opool.tile([128, 2048], bf16, name="osb", tag="osb")
                    nc.scalar.activation(osb, pt, ident, bias=bia[co][:, 0:1])
                    nc.sync.dma_start(
                        out=out_d[co, :, h, half * 2048:(half + 1) * 2048],
                        in_=osb)

        # --- software pipeline: phase h issues mm1(h+4), conv(h+1), mm2(h)
        # so PE's mm2 only depends on the PREVIOUS phase's DVE output.
        for p in range(4):
            for ct in range(2):
                get_x(p, ct)
        mm1(0)
        mm1(1)
        mm1(2)
        conv(0)
        mm1(3)

        for h in range(HP_):
            p = h + 4
            if p < HS_:
                for ct in range(2):
                    get_x(p, ct)
                mm1(p)
            if h + 1 < HP_:
                conv(h + 1)
            mm2(h)

        for pool in (ps, opool, zpool, gpool, tpool, ypool, xpool, consts):
            pool.release()

    nc.compile()
    return nc


def _host_prep(x, U_kh, U_kw, U_kd, U_c_in, U_c_out, bias):
    """Build per-core input maps (numpy only)."""
    x = np.asarray(x)
    U_kh = np.asarray(U_kh, np.float32)
    U_kw = np.asarray(U_kw, np.float32)
    U_kd = np.asarray(U_kd, np.float32)
    U_c_in = np.asarray(U_c_in, np.float32)
    U_c_out = np.asarray(U_c_out, np.float32)
    bias = np.asarray(bias, np.float32)

    xb = np.ascontiguousarray(x[0]).astype(BF16)          # [C, S, S, S]
    xb = xb.reshape(C, S, PLANE)

    # mm1 weights: U_c_in blocks [ct, rt, 128, 128]
    w1 = np.ascontiguousarray(
        U_c_in.astype(BF16).reshape(2, 128, 2, 128).transpose(0, 2, 1, 3))

    # mm2 weights with conv_d taps + U1h*U1w rescale folded in:
    # W_k[r, co] = U_kh[1,r]*U_kw[1,r]*U_kd[k,r]*U_c_out[r,co]
    w2 = np.empty((3, 2, 2, 128, 128), BF16)
    scale_r = U_kh[1] * U_kw[1]                            # [R]
    for k in range(3):
        wk = (scale_r * U_kd[k])[:, None] * U_c_out        # [R, CO]
        w2[k] = wk.astype(BF16).reshape(2, 128, 2, 128).transpose(0, 2, 1, 3)

    # tap ratios for the STT conv passes
    rh = np.stack([U_kh[0] / U_kh[1], U_kh[2] / U_kh[1]], axis=1)  # [R, 2]
    rw = np.stack([U_kw[0] / U_kw[1], U_kw[2] / U_kw[1]], axis=1)
    rh = np.ascontiguousarray(rh.reshape(2, 128, 2).astype(np.float32))
    rw = np.ascontiguousarray(rw.reshape(2, 128, 2).astype(np.float32))
    bias_t = np.ascontiguousarray(bias.reshape(2, 128, 1))

    in_maps = []
    for c in range(NCORES):
        slab = np.zeros((C, HS, PLANE), BF16)
        lo, hi = c * HP - 1, c * HP + HP + 1
        s0, s1 = max(lo, 0), min(hi, S)
        slab[:, s0 - lo:HS - (hi - s1)] = xb[:, s0:s1]
        slab = np.ascontiguousarray(slab.reshape(2, 128, HS, PLANE))
        in_maps.append({
            "xs": slab, "w1": w1, "w2": w2, "rh": rh, "rw": rw,
            "bias_t": bias_t,
        })
    return in_maps


def kernel(x, U_kh, U_kw, U_kd, U_c_in, U_c_out, bias, _trace=False):
    from concourse.bass_utils import run_bass_kernel_spmd

    if "nc" not in _cache:
        _cache["nc"] = _build_program()
    nc = _cache["nc"]

    in_maps = _host_prep(x, U_kh, U_kw, U_kd, U_c_in, U_c_out, bias)
    res = run_bass_kernel_spmd(nc, in_maps, core_ids=list(range(NCORES)),
                               trace=_trace)
    _cache["last_result"] = res

    out = np.empty((1, CO, S, S, S), np.float32)
    for c in range(NCORES):
        o = res.results[c]["out"]                        # [2, 128, HP, PLANE] bf16
        out[0, :, c * HP:(c + 1) * HP] = o.astype(np.float32).reshape(CO, HP, S, S)
    return out


# revision 21
# speedup vs baseline: 1.2267x; 1.0029x over previous
"""Low-rank (CPD) 3D conv kernel for Trainium2, SPMD across 8 NeuronCores.

Math (per reference):
  y[r,h,w,d]  = sum_c U_c_in[c,r] * x[c,h,w,d]
  z           = conv_h(conv_w(conv_d(y)))   (separable 3-tap, per-rank taps)
  out[c,...]  = sum_r U_c_out[r,c] * z[r,...] + bias[c]

Distribution: data-parallel split of H (64) into 8 slabs of 8 planes; each
core reads its slab plus one halo plane on each side (zero at global edges)
and computes its output slab independently. No collectives.

Per-core pipeline (software-pipelined over planes, p = h + 3):
  - mm1 per INPUT plane (1x flops; the old kernel folded conv_h here at 3x):
    y[p] = U_c_in^T x[p], PSUM accumulated over 2 c-tiles, ACT-drained
    dense to bf16.
  - conv_h on DVE: 2 fused scalar_tensor_tensor passes per rank-tile using
    tap ratios U0/U1, U2/U1 (the U1 scale is folded into the mm2 weights),
    full-plane aligned -> 2x DVE mode.
  - conv_w on DVE: same 2-pass STT trick with +-1 w-line shifts (aligned),
    writing into a zero-padded z layout (66-wide d-lines, data in [0:64),
    pads stay zero) so mm2 can read d-shifted views safely.
  - conv_d is folded into mm2: out = sum_{k,rt} W_k[rt] @ z_shift(k) where
    W_k = U_kh[1]*U_kw[1]*U_kd[k] * U_c_out and z_shift(k) is a strided AP
    at element offset k-1 into the padded z lines (PE reads are
    alignment-insensitive; the padding zeros implement d-edge zero-pad).
  - mm2 drain on ACT with per-partition bias, bf16 output (host upcasts to
    f32), halving output DMA.
"""

import numpy as np
import ml_dtypes

BF16 = ml_dtypes.bfloat16

# Problem constants (hardcoded per contest contract)
C = 256   # input channels
R = 256   # rank
CO = 256  # output channels
S = 64    # spatial extent (cube)
NCORES = 8
HP = S // NCORES          # output planes per core (8)
HS = HP + 2               # slab planes incl. halo (10)
PLANE = S * S             # 4096 elements per (w,d) plane
ZLINE = S + 2             # padded d-line length (66)
ZPAD = 2 + ZLINE * S + 2  # padded z tile free dim (guards + 64 lines)

_cache = {}


def _build_program(hp=HP):
    """Build and compile the per-core Bass program (identical on all cores)."""
    import concourse.bass as bass
    import concourse.mybir as mybir
    import concourse.tile as tile
    from concourse import bacc

    HS_, HP_ = hp + 2, hp

    fp32 = mybir.dt.float32
    bf16 = mybir.dt.bfloat16
    mult = mybir.AluOpType.mult
    add = mybir.AluOpType.add
    ident = mybir.ActivationFunctionType.Identity

    nc = bacc.Bacc("TRN2", target_bir_lowering=False, debug=False,
                   num_devices=NCORES)

    # DRAM tensors (names are the in_map keys)
    x_d = nc.dram_tensor("xs", [2, 128, HS_, PLANE], bf16, kind="ExternalInput").ap()
    w1_d = nc.dram_tensor("w1", [2, 2, 128, 128], bf16, kind="ExternalInput").ap()
    w2_d = nc.dram_tensor("w2", [3, 2, 2, 128, 128], bf16, kind="ExternalInput").ap()
    rh_d = nc.dram_tensor("rh", [2, 128, 2], fp32, kind="ExternalInput").ap()
    rw_d = nc.dram_tensor("rw", [2, 128, 2], fp32, kind="ExternalInput").ap()
    bias_d = nc.dram_tensor("bias_t", [2, 128, 1], fp32, kind="ExternalInput").ap()
    out_d = nc.dram_tensor("out", [2, 128, HP_, PLANE], bf16, kind="ExternalOutput").ap()

    with tile.TileContext(nc) as tc:
        consts = tc.alloc_tile_pool(name="consts", bufs=1)
        xpool = tc.alloc_tile_pool(name="x", bufs=4)
        ypool = tc.alloc_tile_pool(name="y", bufs=8)
        tpool = tc.alloc_tile_pool(name="tmp", bufs=4)
        gpool = tc.alloc_tile_pool(name="gtmp", bufs=4)
        zpool = tc.alloc_tile_pool(name="z", bufs=1)
        opool = tc.alloc_tile_pool(name="osb", bufs=2)
        ps1 = tc.alloc_tile_pool(name="ps1", bufs=2, space="PSUM")
        ps2 = tc.alloc_tile_pool(name="ps2", bufs=2, space="PSUM")

        # ---- x plane streaming (x(0) DMA first: it gates mm1(0)) ----
        xt = {}

        def get_x(p, ct):
            if (p, ct) not in xt:
                t = xpool.tile([128, PLANE], bf16, name="xplane", tag="xplane")
                nc.sync.dma_start(out=t, in_=x_d[ct, :, p, :])
                xt[(p, ct)] = t
            return xt[(p, ct)]

        for ct in range(2):
            get_x(0, ct)

        # ---- constants ----
        w1 = [[consts.tile([128, 128], bf16, name=f"w1_{ct}{rt}", tag=f"w1_{ct}{rt}")
               for rt in range(2)] for ct in range(2)]
        for ct in range(2):
            for rt in range(2):
                nc.sync.dma_start(out=w1[ct][rt], in_=w1_d[ct, rt])
        w2 = [[[consts.tile([128, 128], bf16, name=f"w2_{k}{rt}{co}", tag=f"w2_{k}{rt}{co}")
                for co in range(2)] for rt in range(2)] for k in range(3)]
        for k in range(3):
            for rt in range(2):
                for co in range(2):
                    nc.sync.dma_start(out=w2[k][rt][co], in_=w2_d[k, rt, co])
        rh = [consts.tile([128, 2], fp32, name=f"rh{rt}", tag=f"rh{rt}") for rt in range(2)]
        rw = [consts.tile([128, 2], fp32, name=f"rw{rt}", tag=f"rw{rt}") for rt in range(2)]
        bia = [consts.tile([128, 1], fp32, name=f"bias{co}", tag=f"bias{co}") for co in range(2)]
        for rt in range(2):
            nc.sync.dma_start(out=rh[rt], in_=rh_d[rt])
            nc.sync.dma_start(out=rw[rt], in_=rw_d[rt])
        for co in range(2):
            nc.sync.dma_start(out=bia[co], in_=bias_d[co])

        # ---- persistent padded z tiles (pads memset once, stay zero) ----
        zt = {}
        for slot in range(2):
            for rt in range(2):
                t = zpool.tile([128, ZPAD], bf16, name=f"zt{slot}{rt}",
                               tag=f"zt{slot}{rt}")
                # only guards + per-line pad slots need zeroing
                nc.vector.memset(t[:, 0:2], 0.0)
                nc.vector.memset(t[:, ZPAD - 2:ZPAD], 0.0)
                nc.vector.memset(
                    t[:, 2:2 + ZLINE * S].rearrange(
                        "p (w e) -> p w e", e=ZLINE)[:, :, S:ZLINE], 0.0)
                zt[(slot, rt)] = t

        def zlines(slot, rt):
            # [128, 64 lines, 64 data] view of the padded z tile
            return zt[(slot, rt)][:, 2:2 + ZLINE * S].rearrange(
                "p (w e) -> p w e", e=ZLINE)[:, :, 0:S]

        def zrhs(slot, rt, q, k):
            # mm2 moving operand: 8 w-lines x 64 cols at d-offset (k-1)
            b = 2 + ZLINE * (8 * q) + (k - 1)
            return zt[(slot, rt)][:, b:b + 8 * ZLINE].rearrange(
                "p (w e) -> p w e", e=ZLINE)[:, :, 0:S]

        yt = {}  # (p%4, rt) -> dense bf16 y tile

        def mm1(p):
            for rt in range(2):
                if (p % 4, rt) not in yt:
                    yt[(p % 4, rt)] = ypool.tile([128, PLANE], bf16,
                                                 name="yplane", tag="yplane")
                ysb = yt[(p % 4, rt)]
                for qq in range(4):
                    pt = ps1.tile([128, 1024], fp32, name="pt", tag="ps1")
                    for c2 in range(2):
                        q = qq * 2 + c2
                        for ct in range(2):
                            nc.tensor.matmul(
                                pt[:, c2 * 512:(c2 + 1) * 512],
                                w1[ct][rt],
                                get_x(p, ct)[:, q * 512:(q + 1) * 512],
                                start=(ct == 0),
                                stop=(ct == 1),
                            )
                    nc.scalar.copy(ysb[:, qq * 1024:(qq + 1) * 1024], pt)

        def conv(h):
            """conv_h + conv_w for out-plane h -> padded z[(h%2, rt)].

            STT measured 1x on HW (scalar AP costs a read port), so each
            pass is TS (4x) + TT (2x). rt1 runs first and its final z-pass
            goes to GpSimd: that output is consumed only by PE's mm2 next
            phase, so the slow engine has a full phase of slack and never
            blocks the DVE queue. Its inputs live in a dedicated pool so
            DVE tmp-slot recycling doesn't wait on GpSimd either.
            """
            slot = h % 2
            y = {(i, rt): yt[((h + i) % 4, rt)]
                 for i in range(3) for rt in range(2)}
            # rt1's conv_h add goes to GpSimd (dense op only — strided ops
            # are pathological there). Its TS input is DVE's first op, so
            # GpSimd starts ~1us in and finishes before DVE's rt0 chain
            # (13us) reaches the dependent rt1 add.
            sy0_1 = gpool.tile([128, PLANE], bf16, name="sy0_1", tag="gtmp")
            nc.vector.tensor_scalar_mul(sy0_1, y[(0, 1)], rh[1][:, 0:1])
            th1 = gpool.tile([128, PLANE], bf16, name="th1", tag="gtmp")
            nc.gpsimd.tensor_tensor(th1, sy0_1, y[(1, 1)], add)
            for rt in (0, 1):
                if rt == 0:
                    sy0 = tpool.tile([128, PLANE], bf16, name="sy0", tag="tmp")
                    nc.vector.tensor_scalar_mul(sy0, y[(0, 0)], rh[0][:, 0:1])
                    th = tpool.tile([128, PLANE], bf16, name="th", tag="tmp")
                    nc.vector.tensor_tensor(th, sy0, y[(1, 0)], add)
                else:
                    th = th1
                sy2 = tpool.tile([128, PLANE], bf16, name="sy2", tag="tmp")
                nc.vector.tensor_scalar_mul(sy2, y[(2, rt)], rh[rt][:, 1:2])
                a = tpool.tile([128, PLANE], bf16, name="ah", tag="tmp")
                nc.vector.tensor_tensor(a, sy2, th, add)
                av = a.rearrange("p (w q) -> p w q", q=S)

                # conv_w: t2[w] = r0w*a[w-1] + a[w]; z = r2w*a[w+1] + t2
                sa = tpool.tile([128, PLANE], bf16, name="sa", tag="tmp")
                nc.vector.tensor_scalar_mul(sa, a, rw[rt][:, 0:1])
                sav = sa.rearrange("p (w q) -> p w q", q=S)
                t2 = tpool.tile([128, PLANE], bf16, name="t2", tag="tmp")
                t2v = t2.rearrange("p (w q) -> p w q", q=S)
                nc.vector.tensor_tensor(
                    t2v[:, 1:, :], sav[:, :-1, :], av[:, 1:, :], add)
                nc.vector.tensor_copy(t2v[:, 0, :], av[:, 0, :])
                sa2 = tpool.tile([128, PLANE], bf16, name="sa2", tag="tmp")
                nc.vector.tensor_scalar_mul(sa2, a, rw[rt][:, 1:2])
                sa2v = sa2.rearrange("p (w q) -> p w q", q=S)
                zv = zlines(slot, rt)
                nc.vector.tensor_tensor(
                    zv[:, 0:S - 1, :], sa2v[:, 1:, :], t2v[:, 0:S - 1, :], add)
                nc.vector.tensor_copy(zv[:, S - 1, :], t2v[:, S - 1, :])

        def mm2(h):
            slot = h % 2
            for co in range(2):
                for qq in range(4):
                    pt = ps2.tile([128, 1024], fp32, name="pt2", tag="ps2")
                    for c2 in range(2):
                        q = qq * 2 + c2
                        n = 0
                        for rt in range(2):
                            for k in range(3):
                                nc.tensor.matmul(
                                    pt[:, c2 * 512:(c2 + 1) * 512],
                                    w2[k][rt][co],
                                    zrhs(slot, rt, q, k),
                                    start=(n == 0),
                                    stop=(n == 5),
                                )
                                n += 1
                    osb = opool.tile([128, 1024], bf16, name="osb", tag="osb")
                    nc.scalar.activation(osb, pt, ident, bias=bia[co][:, 0:1])
                    nc.sync.dma_start(
                        out=out_d[co, :, h, qq * 1024:(qq + 1) * 1024],
                        in_=osb)

        # --- software pipeline: phase h issues mm1(h+4), conv(h+1), mm2(h)
        # so PE's mm2 only depends on the PREVIOUS phase's DVE output.
        for p in range(4):
            for ct in range(2):
                get_x(p, ct)
        mm1(0)
        mm1(1)
        mm1(2)
        conv(0)
        mm1(3)

        for h in range(HP_):
            p = h + 4
            if p < HS_:
                for ct in range(2):
                    get_x(p, ct)
                mm1(p)
            if h + 1 < HP_:
                conv(h + 1)
            mm2(h)

        for pool in (ps2, ps1, opool, zpool, gpool, tpool, ypool, xpool, consts):
            pool.release()

    nc.compile()
    return nc


def _host_prep(x, U_kh, U_kw, U_kd, U_c_in, U_c_out, bias):
    """Build per-core input maps (numpy only)."""
    x = np.asarray(x)
    U_kh = np.asarray(U_kh, np.float32)
    U_kw = np.asarray(U_kw, np.float32)
    U_kd = np.asarray(U_kd, np.float32)
    U_c_in = np.asarray(U_c_in, np.float32)
    U_c_out = np.asarray(U_c_out, np.float32)
    bias = np.asarray(bias, np.float32)

    xb = np.ascontiguousarray(x[0]).astype(BF16)          # [C, S, S, S]
    xb = xb.reshape(C, S, PLANE)

    # mm1 weights: U_c_in blocks [ct, rt, 128, 128]
    w1 = np.ascontiguousarray(
        U_c_in.astype(BF16).reshape(2, 128, 2, 128).transpose(0, 2, 1, 3))

    # mm2 weights with conv_d taps + U1h*U1w rescale folded in:
    # W_k[r, co] = U_kh[1,r]*U_kw[1,r]*U_kd[k,r]*U_c_out[r,co]
    w2 = np.empty((3, 2, 2, 128, 128), BF16)
    scale_r = U_kh[1] * U_kw[1]                            # [R]
    for k in range(3):
        wk = (scale_r * U_kd[k])[:, None] * U_c_out        # [R, CO]
        w2[k] = wk.astype(BF16).reshape(2, 128, 2, 128).transpose(0, 2, 1, 3)

    # tap ratios for the STT conv passes
    rh = np.stack([U_kh[0] / U_kh[1], U_kh[2] / U_kh[1]], axis=1)  # [R, 2]
    rw = np.stack([U_kw[0] / U_kw[1], U_kw[2] / U_kw[1]], axis=1)
    rh = np.ascontiguousarray(rh.reshape(2, 128, 2).astype(np.float32))
    rw = np.ascontiguousarray(rw.reshape(2, 128, 2).astype(np.float32))
    bias_t = np.ascontiguousarray(bias.reshape(2, 128, 1))

    in_maps = []
    for c in range(NCORES):
        slab = np.zeros((C, HS, PLANE), BF16)
        lo, hi = c * HP - 1, c * HP + HP + 1
        s0, s1 = max(lo, 0), min(hi, S)
        slab[:, s0 - lo:HS - (hi - s1)] = xb[:, s0:s1]
        slab = np.ascontiguousarray(slab.reshape(2, 128, HS, PLANE))
        in_maps.append({
            "xs": slab, "w1": w1, "w2": w2, "rh": rh, "rw": rw,
            "bias_t": bias_t,
        })
    return in_maps


def kernel(x, U_kh, U_kw, U_kd, U_c_in, U_c_out, bias, _trace=False):
    from concourse.bass_utils import run_bass_kernel_spmd

    if "nc" not in _cache:
        _cache["nc"] = _build_program()
    nc = _cache["nc"]

    in_maps = _host_prep(x, U_kh, U_kw, U_kd, U_c_in, U_c_out, bias)
    res = run_bass_kernel_spmd(nc, in_maps, core_ids=list(range(NCORES)),
                               trace=_trace)
    _cache["last_result"] = res

    out = np.empty((1, CO, S, S, S), np.float32)
    for c in range(NCORES):
        o = res.results[c]["out"]                        # [2, 128, HP, PLANE] bf16
        out[0, :, c * HP:(c + 1) * HP] = o.astype(np.float32).reshape(CO, HP, S, S)
    return out
